# revision 1
# baseline (speedup 1.0000x reference)
"""Trainium2 Bass kernel for the grouped contrastive loss.

Math: for anchors i and positives j restricted to the same
sensitive-attribute group g (size P),
    row(i,j) = S_ij - D * log E_ij
with S_ij = <p_i, p_j>/t and E_ij = sum_d exp(p_i[d] p_j[d] / t)
(the log-softmax max-shift cancels analytically), and
    loss = sum_i -1/(N P_i^2) * sum_{j in g(i)} row(i,j).

Strategy: sort points by group host-side so the same-group mask becomes
dense per-group blocks. Work = slots, each slot = (block of <=128 sorted
anchors, j-window of <=W columns of that block's group). Per slot, on
device (anchors on partitions as 32 packs of 4 anchors x 32 dims):
  - S via one fp32 matmul (lhsT = anchor points [32,128], rhs = window
    points [32,W]).
  - E via: DVE tensor_scalar broadcast-multiply (per-pack scalar column
    against 4x-replicated window points), ACT exp (batched 8 packs), and
    per-pack bf16 matmuls against shifted block-diagonal ones that
    accumulate the 32 exp rows of each anchor into its PSUM row.
  - Ln on ACT with accum_out gives sum_j log E per anchor for free.
Dummy rows/columns are weighted out host-side (w=0) or corrected by the
exact constant D*ln(D)*n_dummy per slot. The 8 cores run one SPMD
program over per-core input arrays; each returns a [128] partial that the
host sums.
"""

import math
import os
import sys

sys.path.insert(0, "/opt/trn_rl_repo")

import numpy as np
import ml_dtypes

import concourse.bacc as bacc
import concourse.bass as bass
import concourse.tile as tile
from concourse import mybir
from concourse.bass_utils import run_bass_kernel_spmd

N_CORES = 8
D = 32
PACKS = 32  # packs of 4 anchors per 128-anchor block

last_run_info = {}


def _install_drain_split_patch():
    # This walrus build rejects Drain instructions carrying more than one
    # semaphore wait ("Too many sync wait commands"). TileContext's exit
    # emits one kernel-tail Drain with a wait per outstanding logical
    # processor; split the extras across additional single-wait Drains on
    # the same engine (sequential waits are semantically identical).
    import concourse.tile as tile_mod

    if getattr(tile_mod.TileContext, "_drain_split_patched", False):
        return

    def _drain_and_barrier(self, tick_clock, wait_clock):
        nc = self.nc
        drain_inst = nc.sync.drain()
        wait_clock.add_sem_waits(
            drain_inst.ins,
            tile_mod.ScopedClock({None: tick_clock.global_clock}),
        )
        si = drain_inst.ins.sync_info
        if si is not None and si.on_wait is not None and len(si.on_wait) > 1:
            waits = list(si.on_wait)
            si.on_wait = [waits[0]]
            for w in waits[1:]:
                d2 = nc.sync.drain()
                si2 = d2.ins.sync_info
                if si2 is None:
                    d2.ins.sync_info = type(si)(on_wait=[w], on_update=[])
                else:
                    si2.on_wait = [w]

        nc.all_engine_barrier()
        assert self.sems is not None
        popped = nc._tile_sem_poison_stack.pop()
        assert popped is self._sem_poison
        nc.clear_and_free_semaphores(list(self.sems.allocated().values()))
        nc.all_engine_barrier()

    tile_mod.TileContext._drain_and_barrier = _drain_and_barrier
    tile_mod.TileContext._drain_split_patched = True


def _install_ntff_hook():
    # bass_utils' trace path under axon imports antenv.axon_hooks, which is
    # absent in this image; provide the ctypes-based hook it expects.
    import contextlib
    import ctypes
    import types

    if "antenv.axon_hooks" in sys.modules:
        return

    def _make_hook():
        try:
            lib = ctypes.CDLL("/opt/axon/libaxon_pjrt.so")
        except OSError:
            return None
        if not hasattr(lib, "axon_start_nrt_profile"):
            return None
        lib.axon_start_nrt_profile.argtypes = [
            ctypes.POINTER(ctypes.c_int64),
            ctypes.c_size_t,
        ]
        lib.axon_start_nrt_profile.restype = ctypes.c_int64
        lib.axon_stop_nrt_profile.argtypes = [ctypes.c_char_p]
        lib.axon_stop_nrt_profile.restype = ctypes.c_int64

        @contextlib.contextmanager
        def _hook_cm(output_dir, device_ids):
            import jax

            jax.devices()
            if device_ids:
                ids = (ctypes.c_int64 * len(device_ids))(*device_ids)
                rc = lib.axon_start_nrt_profile(ids, len(device_ids))
            else:
                rc = lib.axon_start_nrt_profile(None, 0)
            if rc != 0:
                raise RuntimeError(f"axon_start_nrt_profile rc={rc}")
            try:
                yield
            finally:
                n = lib.axon_stop_nrt_profile(str(output_dir).encode())
                if n < 0:
                    raise RuntimeError(f"axon_stop_nrt_profile rc={n}")

        return _hook_cm

    hook = _make_hook()
    mod = types.ModuleType("antenv.axon_hooks")
    mod.get_axon_ntff_profile_hook = lambda: hook
    mod.set_axon_ntff_profile_hook = lambda h: None
    sys.modules["antenv.axon_hooks"] = mod


def _plan(sa_sorted):
    """Slot plan from the sorted attribute vector.

    Returns (W, ntiles, slots_per_core) where each slot is
    (pos0, row_lo, row_hi, g0, g1, c0, L):
      pos0: first sorted-anchor position of the 128-anchor block
      [row_lo, row_hi): rows of the block belonging to group [g0, g1)
      [c0, c0+L): this slot's j-window (sorted positions) within the group
    or None for a dummy slot.
    """
    n = len(sa_sorted)
    assert n % 128 == 0
    n_blocks = n // 128
    bounds = [0]
    for i in range(1, n):
        if sa_sorted[i] != sa_sorted[i - 1]:
            bounds.append(i)
    bounds.append(n)

    jobs = []  # (pos0, row_lo, row_hi, g0, g1)
    for b in range(n_blocks):
        pos0 = b * 128
        for gi in range(len(bounds) - 1):
            g0, g1 = bounds[gi], bounds[gi + 1]
            lo = max(pos0, g0)
            hi = min(pos0 + 128, g1)
            if lo < hi:
                jobs.append((pos0, lo - pos0, hi - pos0, g0, g1))

    best = None
    for W in range(128, 513, 16):
        T = sum((g1 - g0 + W - 1) // W for (_, _, _, g0, g1) in jobs)
        ntiles = (T + N_CORES - 1) // N_CORES
        cost = ntiles * W
        if best is None or cost < best[0] or (cost == best[0] and W > best[1]):
            best = (cost, W, ntiles)
    _, W, ntiles = best

    slots = []
    for pos0, row_lo, row_hi, g0, g1 in jobs:
        for c0 in range(g0, g1, W):
            L = min(W, g1 - c0)
            slots.append((pos0, row_lo, row_hi, g0, g1, c0, L))

    per_core = [[] for _ in range(N_CORES)]
    for i, s in enumerate(slots):
        per_core[i % N_CORES].append(s)
    for c in range(N_CORES):
        while len(per_core[c]) < ntiles:
            per_core[c].append(None)
    return W, ntiles, per_core


def _build_program(W, ntiles):
    # Bacc (not raw Bass): its compile() runs generate_event_semaphores,
    # which splits multi-semaphore waits to satisfy the TRN2 one-wait-per-
    # instruction constraint this walrus build enforces.
    nc = bacc.Bacc(
        "TRN2", target_bir_lowering=False, debug=False, num_devices=N_CORES
    )
    f32 = mybir.dt.float32
    bf16 = mybir.dt.bfloat16

    rep4_d = nc.dram_tensor("rep4", [128, ntiles * W], f32, kind="ExternalInput").ap()
    rhsj_d = nc.dram_tensor("rhsj", [32, ntiles * W], f32, kind="ExternalInput").ap()
    lhsa_d = nc.dram_tensor("lhsa", [32, ntiles * 128], f32, kind="ExternalInput").ap()
    scal_d = nc.dram_tensor("scal", [128, ntiles * PACKS], f32, kind="ExternalInput").ap()
    wcol_d = nc.dram_tensor("wcol", [128, ntiles], f32, kind="ExternalInput").ap()
    kcol_d = nc.dram_tensor("kcol", [128, ntiles], f32, kind="ExternalInput").ap()
    ones_d = nc.dram_tensor("onesbd", [128, 8 * 32], bf16, kind="ExternalInput").ap()
    out_d = nc.dram_tensor("out", [128, 1], f32, kind="ExternalOutput").ap()

    Exp = mybir.ActivationFunctionType.Exp
    Ln = mybir.ActivationFunctionType.Ln

    with tile.TileContext(nc) as tc:
        with (
            tc.tile_pool(name="const", bufs=1) as cpool,
            tc.tile_pool(name="work", bufs=3) as wpool,
            tc.tile_pool(name="red", bufs=2) as rpool,
            tc.tile_pool(name="psE", bufs=2, space="PSUM") as psE,
            tc.tile_pool(name="psS", bufs=2, space="PSUM") as psS,
            tc.tile_pool(name="psL", bufs=1, space="PSUM") as psL,
        ):
            rep4 = cpool.tile([128, ntiles * W], f32, tag="rep4")
            nc.gpsimd.dma_start(rep4[:], rep4_d[:])
            rhsj = cpool.tile([32, ntiles * W], f32, tag="rhsj")
            nc.gpsimd.dma_start(rhsj[:], rhsj_d[:])
            lhsa = cpool.tile([32, ntiles * 128], f32, tag="lhsa")
            nc.gpsimd.dma_start(lhsa[:], lhsa_d[:])
            scal = cpool.tile([128, ntiles * PACKS], f32, tag="scal")
            nc.gpsimd.dma_start(scal[:], scal_d[:])
            wcol = cpool.tile([128, ntiles], f32, tag="wcol")
            nc.gpsimd.dma_start(wcol[:], wcol_d[:])
            kcol = cpool.tile([128, ntiles], f32, tag="kcol")
            nc.gpsimd.dma_start(kcol[:], kcol_d[:])
            onesbd = cpool.tile([128, 8 * 32], bf16, tag="onesbd")
            nc.gpsimd.dma_start(onesbd[:], ones_d[:])

            acc = cpool.tile([128, 1], f32, tag="acc")
            nc.vector.memset(acc[:], 0.0)

            for s in range(ntiles):
                S_ps = psS.tile([128, W], f32, tag="S")
                nc.tensor.matmul(
                    S_ps[:],
                    lhsT=lhsa[:, s * 128 : (s + 1) * 128],
                    rhs=rhsj[:, s * W : (s + 1) * W],
                    start=True,
                    stop=True,
                )
                # PSUM APs can only start at partition 0/32/64, so the 128
                # anchor rows of E live in two [64, W] tiles.
                E_lo = psE.tile([64, W], f32, tag="Elo")
                E_hi = psE.tile([64, W], f32, tag="Ehi")
                for h in range(4):
                    prod = wpool.tile([128, 8 * W], f32, tag="prod")
                    for i in range(8):
                        k = 8 * h + i
                        nc.vector.tensor_scalar_mul(
                            prod[:, i * W : (i + 1) * W],
                            rep4[:, s * W : (s + 1) * W],
                            scal[:, s * PACKS + k : s * PACKS + k + 1],
                        )
                    expt = wpool.tile([128, 8 * W], bf16, tag="expt")
                    nc.scalar.activation(expt[:], prod[:], Exp)
                    E_t = E_lo if h < 2 else E_hi
                    rb = 32 * (h % 2)
                    for i in range(8):
                        nc.tensor.matmul(
                            E_t[rb : rb + 32, :],
                            lhsT=onesbd[:, 32 * i : 32 * (i + 1)],
                            rhs=expt[:, i * W : (i + 1) * W],
                            start=(i == 0),
                            stop=(i == 7),
                        )
                logE = psL.tile([128, W], f32, tag="logE")
                sL = rpool.tile([128, 1], f32, tag="sL")
                nc.scalar.activation(logE[0:64, :], E_lo[:], Ln, accum_out=sL[0:64, :])
                nc.scalar.activation(logE[64:128, :], E_hi[:], Ln, accum_out=sL[64:128, :])
                sS = rpool.tile([128, 1], f32, tag="sS")
                nc.vector.tensor_reduce(
                    sS[:], S_ps[:], axis=mybir.AxisListType.X, op=mybir.AluOpType.add
                )
                v1 = rpool.tile([128, 1], f32, tag="v1")
                nc.vector.tensor_scalar(
                    v1[:],
                    sL[:],
                    -float(D),
                    kcol[:, s : s + 1],
                    op0=mybir.AluOpType.mult,
                    op1=mybir.AluOpType.add,
                )
                v2 = rpool.tile([128, 1], f32, tag="v2")
                nc.vector.tensor_add(v2[:], v1[:], sS[:])
                nc.vector.scalar_tensor_tensor(
                    acc[:],
                    v2[:],
                    wcol[:, s : s + 1],
                    acc[:],
                    op0=mybir.AluOpType.mult,
                    op1=mybir.AluOpType.add,
                )

            nc.gpsimd.dma_start(out_d[:], acc[:])

    nc.compile()
    return nc


def kernel(points, sensitive_attribute, t):
    _install_ntff_hook()

    points = np.asarray(points, dtype=np.float32)
    sa = np.asarray(sensitive_attribute).astype(np.int64)
    n, d = points.shape
    assert d == D

    scale = 1.0 / math.sqrt(float(np.asarray(t)))
    order = np.argsort(sa, kind="stable")
    sa_sorted = sa[order]
    ps = (points[order] * np.float32(scale)).astype(np.float32)  # [n, 32] sorted

    W, ntiles, per_core = _plan(sa_sorted)

    lnD = math.log(float(D))
    in_maps = []
    for c in range(N_CORES):
        rep4 = np.zeros((128, ntiles * W), np.float32)
        rhsj = np.zeros((32, ntiles * W), np.float32)
        lhsa = np.zeros((32, ntiles * 128), np.float32)
        scal = np.zeros((128, ntiles * PACKS), np.float32)
        wcol = np.zeros((128, ntiles), np.float32)
        kcol = np.zeros((128, ntiles), np.float32)
        for s, slot in enumerate(per_core[c]):
            if slot is None:
                # dummy slot: all-zero data; exp(0) rows sum to D, finite
                # log, zero weight. Correction value irrelevant (w=0).
                continue
            pos0, row_lo, row_hi, g0, g1, c0, L = slot
            P = g1 - g0
            win = ps[c0 : c0 + L].T  # [32, L]
            rhsj[:, s * W : s * W + L] = win
            rep4[:, s * W : s * W + L] = np.tile(win, (4, 1))
            ablk = np.zeros((32, 128), np.float32)
            ablk[:, row_lo:row_hi] = ps[pos0 + row_lo : pos0 + row_hi].T
            lhsa[:, s * 128 : (s + 1) * 128] = ablk
            # scal column k = anchors 4k..4k+3 flattened (a-major, d-minor)
            scal[:, s * PACKS : (s + 1) * PACKS] = (
                ablk.T.reshape(PACKS, 128).T
            )
            wcol[row_lo:row_hi, s] = -1.0 / (n * float(P) * float(P))
            kcol[:, s] = D * lnD * (W - L)

        onesbd = np.zeros((128, 8 * 32), ml_dtypes.bfloat16)
        for r in range(8):
            for a in range(4):
                onesbd[32 * a : 32 * (a + 1), 32 * r + 4 * r + a] = 1.0
        in_maps.append(
            {
                "rep4": rep4,
                "rhsj": rhsj,
                "lhsa": lhsa,
                "scal": scal,
                "wcol": wcol,
                "kcol": kcol,
                "onesbd": onesbd,
            }
        )

    nc = _build_program(W, ntiles)
    trace = bool(int(os.environ.get("KERNEL_TRACE", "0")))
    res = run_bass_kernel_spmd(nc, in_maps, list(range(N_CORES)), trace=trace)
    last_run_info["exec_time_ns"] = res.exec_time_ns
    last_run_info["mean_exec_time_ns"] = res.mean_exec_time_ns
    last_run_info["W"] = W
    last_run_info["ntiles"] = ntiles
    last_run_info["instructions"] = (
        res.instructions_and_trace[0] if res.instructions_and_trace else None
    )

    total = 0.0
    for c in range(N_CORES):
        total += float(res.results[c]["out"].astype(np.float64).sum())
    return np.float32(total)



# revision 3
# speedup vs baseline: 1.3270x; 1.3270x over previous
"""Trainium2 Bass kernel for the grouped contrastive loss.

Math: for anchors i and positives j restricted to the same
sensitive-attribute group g (size P),
    row(i,j) = S_ij - D * log E_ij
with S_ij = <p_i, p_j>/t and E_ij = sum_d exp(p_i[d] p_j[d] / t)
(the log-softmax max-shift cancels analytically), and
    loss = sum_i -1/(N P_i^2) * sum_{j in g(i)} row(i,j).

Strategy: sort points by group host-side so the same-group mask becomes
dense per-group blocks. Work = slots, each slot = (block of <=128 sorted
anchors, j-window of <=W columns of that block's group). Per slot, on
device (anchors on partitions as 32 packs of 4 anchors x 32 dims):
  - S via one fp32 matmul (lhsT = anchor points [32,128], rhs = window
    points [32,W]).
  - E via: DVE tensor_scalar broadcast-multiply (per-pack scalar column
    against 4x-replicated window points), ACT exp (batched 8 packs), and
    per-pack bf16 matmuls against shifted block-diagonal ones that
    accumulate the 32 exp rows of each anchor into its PSUM row.
  - Ln on ACT with accum_out gives sum_j log E per anchor for free.
Dummy rows/columns are weighted out host-side (w=0) or corrected by the
exact constant D*ln(D)*n_dummy per slot. The 8 cores run one SPMD
program over per-core input arrays; each returns a [128] partial that the
host sums.
"""

import math
import os
import sys

sys.path.insert(0, "/opt/trn_rl_repo")

import numpy as np
import ml_dtypes

import concourse.bacc as bacc
import concourse.bass as bass
import concourse.tile as tile
from concourse import mybir
from concourse.bass_utils import run_bass_kernel_spmd

N_CORES = 8
D = 32
PACKS = 32  # packs of 4 anchors per 128-anchor block

last_run_info = {}


def _install_drain_split_patch():
    # This walrus build rejects Drain instructions carrying more than one
    # semaphore wait ("Too many sync wait commands"). TileContext's exit
    # emits one kernel-tail Drain with a wait per outstanding logical
    # processor; split the extras across additional single-wait Drains on
    # the same engine (sequential waits are semantically identical).
    import concourse.tile as tile_mod

    if getattr(tile_mod.TileContext, "_drain_split_patched", False):
        return

    def _drain_and_barrier(self, tick_clock, wait_clock):
        nc = self.nc
        drain_inst = nc.sync.drain()
        wait_clock.add_sem_waits(
            drain_inst.ins,
            tile_mod.ScopedClock({None: tick_clock.global_clock}),
        )
        si = drain_inst.ins.sync_info
        if si is not None and si.on_wait is not None and len(si.on_wait) > 1:
            waits = list(si.on_wait)
            si.on_wait = [waits[0]]
            for w in waits[1:]:
                d2 = nc.sync.drain()
                si2 = d2.ins.sync_info
                if si2 is None:
                    d2.ins.sync_info = type(si)(on_wait=[w], on_update=[])
                else:
                    si2.on_wait = [w]

        nc.all_engine_barrier()
        assert self.sems is not None
        popped = nc._tile_sem_poison_stack.pop()
        assert popped is self._sem_poison
        nc.clear_and_free_semaphores(list(self.sems.allocated().values()))
        nc.all_engine_barrier()

    tile_mod.TileContext._drain_and_barrier = _drain_and_barrier
    tile_mod.TileContext._drain_split_patched = True


def _install_ntff_hook():
    # bass_utils' trace path under axon imports antenv.axon_hooks, which is
    # absent in this image; provide the ctypes-based hook it expects.
    import contextlib
    import ctypes
    import types

    if "antenv.axon_hooks" in sys.modules:
        return

    def _make_hook():
        try:
            lib = ctypes.CDLL("/opt/axon/libaxon_pjrt.so")
        except OSError:
            return None
        if not hasattr(lib, "axon_start_nrt_profile"):
            return None
        lib.axon_start_nrt_profile.argtypes = [
            ctypes.POINTER(ctypes.c_int64),
            ctypes.c_size_t,
        ]
        lib.axon_start_nrt_profile.restype = ctypes.c_int64
        lib.axon_stop_nrt_profile.argtypes = [ctypes.c_char_p]
        lib.axon_stop_nrt_profile.restype = ctypes.c_int64

        @contextlib.contextmanager
        def _hook_cm(output_dir, device_ids):
            import jax

            jax.devices()
            if device_ids:
                ids = (ctypes.c_int64 * len(device_ids))(*device_ids)
                rc = lib.axon_start_nrt_profile(ids, len(device_ids))
            else:
                rc = lib.axon_start_nrt_profile(None, 0)
            if rc != 0:
                raise RuntimeError(f"axon_start_nrt_profile rc={rc}")
            try:
                yield
            finally:
                n = lib.axon_stop_nrt_profile(str(output_dir).encode())
                if n < 0:
                    raise RuntimeError(f"axon_stop_nrt_profile rc={n}")

        return _hook_cm

    hook = _make_hook()
    mod = types.ModuleType("antenv.axon_hooks")
    mod.get_axon_ntff_profile_hook = lambda: hook
    mod.set_axon_ntff_profile_hook = lambda h: None
    sys.modules["antenv.axon_hooks"] = mod


def _install_act_table_patch():
    # The greedy act-table fixpoint picks the first table containing each
    # activation func, so Exp->exp_and_others and Ln->natural_log thrash
    # ACT_TABLE_LOADs (1283ns each) every tile. Mask every set except the
    # combined natural_log_exp_and_others (keeping dict order, hence the
    # act_func_set_id indices, intact) so one table serves both and the
    # load hoists out of the loop.
    import concourse.hw_specs as hw_specs
    import concourse.bass_interp as bass_interp

    if getattr(bacc, "_act_table_patched", False):
        return
    orig = hw_specs.get_activation_tables

    def patched(arch):
        t = orig(arch)
        keep = "natural_log_exp_and_others"
        if keep not in t:
            return t
        return {k: (v if k == keep else set()) for k, v in t.items()}

    bacc.get_activation_tables = patched
    bass_interp.get_activation_tables = patched
    bacc._act_table_patched = True


def _plan(sa_sorted):
    """Slot plan from the sorted attribute vector.

    Returns (W, ntiles, slots_per_core) where each slot is
    (pos0, row_lo, row_hi, g0, g1, c0, L):
      pos0: first sorted-anchor position of the 128-anchor block
      [row_lo, row_hi): rows of the block belonging to group [g0, g1)
      [c0, c0+L): this slot's j-window (sorted positions) within the group
    or None for a dummy slot.
    """
    n = len(sa_sorted)
    assert n % 128 == 0
    n_blocks = n // 128
    bounds = [0]
    for i in range(1, n):
        if sa_sorted[i] != sa_sorted[i - 1]:
            bounds.append(i)
    bounds.append(n)

    jobs = []  # (pos0, row_lo, row_hi, g0, g1)
    for b in range(n_blocks):
        pos0 = b * 128
        for gi in range(len(bounds) - 1):
            g0, g1 = bounds[gi], bounds[gi + 1]
            lo = max(pos0, g0)
            hi = min(pos0 + 128, g1)
            if lo < hi:
                jobs.append((pos0, lo - pos0, hi - pos0, g0, g1))

    best = None
    for W in range(128, 513, 16):
        T = sum((g1 - g0 + W - 1) // W for (_, _, _, g0, g1) in jobs)
        ntiles = (T + N_CORES - 1) // N_CORES
        cost = ntiles * W
        if best is None or cost < best[0] or (cost == best[0] and W > best[1]):
            best = (cost, W, ntiles)
    _, W, ntiles = best

    slots = []
    for pos0, row_lo, row_hi, g0, g1 in jobs:
        for c0 in range(g0, g1, W):
            L = min(W, g1 - c0)
            slots.append((pos0, row_lo, row_hi, g0, g1, c0, L))

    per_core = [[] for _ in range(N_CORES)]
    for i, s in enumerate(slots):
        per_core[i % N_CORES].append(s)
    for c in range(N_CORES):
        while len(per_core[c]) < ntiles:
            per_core[c].append(None)
    return W, ntiles, per_core


def _build_program(W, ntiles):
    # Bacc (not raw Bass): its compile() runs generate_event_semaphores,
    # which splits multi-semaphore waits to satisfy the TRN2 one-wait-per-
    # instruction constraint this walrus build enforces.
    nc = bacc.Bacc(
        "TRN2", target_bir_lowering=False, debug=False, num_devices=N_CORES
    )
    f32 = mybir.dt.float32
    bf16 = mybir.dt.bfloat16

    rep4_d = nc.dram_tensor("rep4", [128, ntiles * W], f32, kind="ExternalInput").ap()
    rhsj_d = nc.dram_tensor("rhsj", [32, ntiles * W], f32, kind="ExternalInput").ap()
    lhsa_d = nc.dram_tensor("lhsa", [32, ntiles * 128], f32, kind="ExternalInput").ap()
    scal_d = nc.dram_tensor("scal", [128, ntiles * PACKS], f32, kind="ExternalInput").ap()
    wcol_d = nc.dram_tensor("wcol", [128, ntiles], f32, kind="ExternalInput").ap()
    kcol_d = nc.dram_tensor("kcol", [128, ntiles], f32, kind="ExternalInput").ap()
    ones_d = nc.dram_tensor("onesbd", [128, 8 * 32], bf16, kind="ExternalInput").ap()
    out_d = nc.dram_tensor("out", [128, 1], f32, kind="ExternalOutput").ap()

    Exp = mybir.ActivationFunctionType.Exp
    Ln = mybir.ActivationFunctionType.Ln

    with tile.TileContext(nc) as tc:
        with (
            tc.tile_pool(name="const", bufs=1) as cpool,
            tc.tile_pool(name="work", bufs=3) as wpool,
            tc.tile_pool(name="red", bufs=2) as rpool,
            tc.tile_pool(name="psE", bufs=2, space="PSUM") as psE,
            tc.tile_pool(name="psS", bufs=2, space="PSUM") as psS,
            tc.tile_pool(name="psL", bufs=1, space="PSUM") as psL,
        ):
            rep4 = cpool.tile([128, ntiles * W], f32, tag="rep4")
            nc.gpsimd.dma_start(rep4[:], rep4_d[:])
            rhsj = cpool.tile([32, ntiles * W], f32, tag="rhsj")
            nc.gpsimd.dma_start(rhsj[:], rhsj_d[:])
            lhsa = cpool.tile([32, ntiles * 128], f32, tag="lhsa")
            nc.gpsimd.dma_start(lhsa[:], lhsa_d[:])
            scal = cpool.tile([128, ntiles * PACKS], f32, tag="scal")
            nc.gpsimd.dma_start(scal[:], scal_d[:])
            wcol = cpool.tile([128, ntiles], f32, tag="wcol")
            nc.gpsimd.dma_start(wcol[:], wcol_d[:])
            kcol = cpool.tile([128, ntiles], f32, tag="kcol")
            nc.gpsimd.dma_start(kcol[:], kcol_d[:])
            onesbd = cpool.tile([128, 8 * 32], bf16, tag="onesbd")
            nc.gpsimd.dma_start(onesbd[:], ones_d[:])

            acc = cpool.tile([128, 1], f32, tag="acc")
            nc.vector.memset(acc[:], 0.0)

            for s in range(ntiles):
                S_ps = psS.tile([128, W], f32, tag="S")
                nc.tensor.matmul(
                    S_ps[:],
                    lhsT=lhsa[:, s * 128 : (s + 1) * 128],
                    rhs=rhsj[:, s * W : (s + 1) * W],
                    start=True,
                    stop=True,
                )
                # PSUM APs can only start at partition 0/32/64, so the 128
                # anchor rows of E live in two [64, W] tiles.
                E_lo = psE.tile([64, W], f32, tag="Elo")
                E_hi = psE.tile([64, W], f32, tag="Ehi")
                for h in range(4):
                    prod = wpool.tile([128, 8 * W], f32, tag="prod")
                    for i in range(8):
                        k = 8 * h + i
                        nc.vector.tensor_scalar_mul(
                            prod[:, i * W : (i + 1) * W],
                            rep4[:, s * W : (s + 1) * W],
                            scal[:, s * PACKS + k : s * PACKS + k + 1],
                        )
                    expt = wpool.tile([128, 8 * W], bf16, tag="expt")
                    nc.scalar.activation(expt[:], prod[:], Exp)
                    E_t = E_lo if h < 2 else E_hi
                    rb = 32 * (h % 2)
                    for i in range(8):
                        nc.tensor.matmul(
                            E_t[rb : rb + 32, :],
                            lhsT=onesbd[:, 32 * i : 32 * (i + 1)],
                            rhs=expt[:, i * W : (i + 1) * W],
                            start=(i == 0),
                            stop=(i == 7),
                        )
                logE = psL.tile([128, W], f32, tag="logE")
                sL = rpool.tile([128, 1], f32, tag="sL")
                nc.scalar.activation(logE[0:64, :], E_lo[:], Ln, accum_out=sL[0:64, :])
                nc.scalar.activation(logE[64:128, :], E_hi[:], Ln, accum_out=sL[64:128, :])
                sS = rpool.tile([128, 1], f32, tag="sS")
                nc.vector.tensor_reduce(
                    sS[:], S_ps[:], axis=mybir.AxisListType.X, op=mybir.AluOpType.add
                )
                v1 = rpool.tile([128, 1], f32, tag="v1")
                nc.vector.tensor_scalar(
                    v1[:],
                    sL[:],
                    -float(D),
                    kcol[:, s : s + 1],
                    op0=mybir.AluOpType.mult,
                    op1=mybir.AluOpType.add,
                )
                v2 = rpool.tile([128, 1], f32, tag="v2")
                nc.vector.tensor_add(v2[:], v1[:], sS[:])
                nc.vector.scalar_tensor_tensor(
                    acc[:],
                    v2[:],
                    wcol[:, s : s + 1],
                    acc[:],
                    op0=mybir.AluOpType.mult,
                    op1=mybir.AluOpType.add,
                )

            nc.gpsimd.dma_start(out_d[:], acc[:])

    nc.compile()
    return nc


def kernel(points, sensitive_attribute, t):
    _install_ntff_hook()
    _install_act_table_patch()

    points = np.asarray(points, dtype=np.float32)
    sa = np.asarray(sensitive_attribute).astype(np.int64)
    n, d = points.shape
    assert d == D

    scale = 1.0 / math.sqrt(float(np.asarray(t)))
    order = np.argsort(sa, kind="stable")
    sa_sorted = sa[order]
    ps = (points[order] * np.float32(scale)).astype(np.float32)  # [n, 32] sorted

    W, ntiles, per_core = _plan(sa_sorted)

    lnD = math.log(float(D))
    in_maps = []
    for c in range(N_CORES):
        rep4 = np.zeros((128, ntiles * W), np.float32)
        rhsj = np.zeros((32, ntiles * W), np.float32)
        lhsa = np.zeros((32, ntiles * 128), np.float32)
        scal = np.zeros((128, ntiles * PACKS), np.float32)
        wcol = np.zeros((128, ntiles), np.float32)
        kcol = np.zeros((128, ntiles), np.float32)
        for s, slot in enumerate(per_core[c]):
            if slot is None:
                # dummy slot: all-zero data; exp(0) rows sum to D, finite
                # log, zero weight. Correction value irrelevant (w=0).
                continue
            pos0, row_lo, row_hi, g0, g1, c0, L = slot
            P = g1 - g0
            win = ps[c0 : c0 + L].T  # [32, L]
            rhsj[:, s * W : s * W + L] = win
            rep4[:, s * W : s * W + L] = np.tile(win, (4, 1))
            ablk = np.zeros((32, 128), np.float32)
            ablk[:, row_lo:row_hi] = ps[pos0 + row_lo : pos0 + row_hi].T
            lhsa[:, s * 128 : (s + 1) * 128] = ablk
            # scal column k = anchors 4k..4k+3 flattened (a-major, d-minor)
            scal[:, s * PACKS : (s + 1) * PACKS] = (
                ablk.T.reshape(PACKS, 128).T
            )
            wcol[row_lo:row_hi, s] = -1.0 / (n * float(P) * float(P))
            kcol[:, s] = D * lnD * (W - L)

        onesbd = np.zeros((128, 8 * 32), ml_dtypes.bfloat16)
        for r in range(8):
            for a in range(4):
                onesbd[32 * a : 32 * (a + 1), 32 * r + 4 * r + a] = 1.0
        in_maps.append(
            {
                "rep4": rep4,
                "rhsj": rhsj,
                "lhsa": lhsa,
                "scal": scal,
                "wcol": wcol,
                "kcol": kcol,
                "onesbd": onesbd,
            }
        )

    nc = _build_program(W, ntiles)
    trace = bool(int(os.environ.get("KERNEL_TRACE", "0")))
    res = run_bass_kernel_spmd(nc, in_maps, list(range(N_CORES)), trace=trace)
    last_run_info["exec_time_ns"] = res.exec_time_ns
    last_run_info["mean_exec_time_ns"] = res.mean_exec_time_ns
    last_run_info["W"] = W
    last_run_info["ntiles"] = ntiles
    last_run_info["instructions"] = (
        res.instructions_and_trace[0] if res.instructions_and_trace else None
    )

    total = 0.0
    for c in range(N_CORES):
        total += float(res.results[c]["out"].astype(np.float64).sum())
    return np.float32(total)



# revision 6
# speedup vs baseline: 2.0301x; 1.5299x over previous
"""Trainium2 Bass kernel for the grouped contrastive loss.

Math: for anchors i and positives j in the same sensitive-attribute group g
(size P), with x_ij_d = p_i[d] p_j[d] / t:
    row(i,j) = S_ij - D * ln E_ij,   S_ij = sum_d x_ij_d,  E_ij = sum_d e^{x_ij_d}
(the log-softmax max-shift cancels analytically), and
    loss = sum_g -1/(N P^2) * sum_{i,j in g} row(i,j).

row is symmetric in (i,j), so after sorting points by group each group is
covered by per-block triangles: for each block B of <=128 consecutive sorted
anchors, process the full B x B square once (weight 1) plus the window
[B_end, g_end) (weight 2).  This does ~55% of the full-square element work
with plain per-row weights (the square covers both triangles + diagonal of
B x B exactly).

Work unit = slot: (job's <=128 anchors, a window piece of <=W cols).  Per
slot, on device (anchors packed 4-per-32-partition-span, dims on partitions):
  - 32 DVE tensor_scalar muls (bf16 in/out, 4x mode) build x for all packs.
  - one ACT Exp over [128, 32W] (bf16).
  - 32 PE matmuls vs a block-diagonal ones matrix accumulate each anchor's
    32 exp rows into its E row in PSUM ([128, W], quadrant cascades).
  - one ACT Ln over [128, W] with accum_out gives sum_j ln E per anchor.
  - sum_j S_ij = <a_i, sum_j w_j> collapses to one tiny PE matmul per slot
    (window column-sums precomputed host-side), weight folded into the
    anchor matrix, accumulated across slots in PSUM.
  - one gpsimd scalar_tensor_tensor folds -D * w * sumlnE into the f32
    accumulator.
Padding columns (zeros) contribute exactly D*ln(D) each to the ln-sum and 0
to the S-sum; the host folds the exact correction into the accumulator init.
All 8 cores run one SPMD program (identical slot-width schedule, per-core
data); each returns a [128] partial that the host sums.
"""

import math
import os
import sys

sys.path.insert(0, "/opt/trn_rl_repo")

import numpy as np
import ml_dtypes

import concourse.bacc as bacc
import concourse.bass as bass
import concourse.tile as tile
from concourse import mybir
from concourse.bass_utils import run_bass_kernel_spmd

N_CORES = 8
D = 32

last_run_info = {}


def _install_ntff_hook():
    # bass_utils' trace path under axon imports antenv.axon_hooks, which is
    # absent in this image; provide the ctypes-based hook it expects.
    import contextlib
    import ctypes
    import types

    if "antenv.axon_hooks" in sys.modules:
        return

    def _make_hook():
        try:
            lib = ctypes.CDLL("/opt/axon/libaxon_pjrt.so")
        except OSError:
            return None
        if not hasattr(lib, "axon_start_nrt_profile"):
            return None
        lib.axon_start_nrt_profile.argtypes = [
            ctypes.POINTER(ctypes.c_int64),
            ctypes.c_size_t,
        ]
        lib.axon_start_nrt_profile.restype = ctypes.c_int64
        lib.axon_stop_nrt_profile.argtypes = [ctypes.c_char_p]
        lib.axon_stop_nrt_profile.restype = ctypes.c_int64

        @contextlib.contextmanager
        def _hook_cm(output_dir, device_ids):
            import jax

            jax.devices()
            if device_ids:
                ids = (ctypes.c_int64 * len(device_ids))(*device_ids)
                rc = lib.axon_start_nrt_profile(ids, len(device_ids))
            else:
                rc = lib.axon_start_nrt_profile(None, 0)
            if rc != 0:
                raise RuntimeError(f"axon_start_nrt_profile rc={rc}")
            try:
                yield
            finally:
                n = lib.axon_stop_nrt_profile(str(output_dir).encode())
                if n < 0:
                    raise RuntimeError(f"axon_stop_nrt_profile rc={n}")

        return _hook_cm

    hook = _make_hook()
    mod = types.ModuleType("antenv.axon_hooks")
    mod.get_axon_ntff_profile_hook = lambda: hook
    mod.set_axon_ntff_profile_hook = lambda h: None
    sys.modules["antenv.axon_hooks"] = mod


def _install_act_table_patch():
    # The greedy act-table fixpoint picks the first table containing each
    # activation func, so Exp->exp_and_others and Ln->natural_log thrash
    # ACT_TABLE_LOADs (1283ns each) every tile. Mask every set except the
    # combined natural_log_exp_and_others (keeping dict order, hence the
    # act_func_set_id indices, intact) so one table serves both and the
    # load hoists out of the loop.
    import concourse.hw_specs as hw_specs
    import concourse.bass_interp as bass_interp

    if getattr(bacc, "_act_table_patched", False):
        return
    orig = hw_specs.get_activation_tables

    def patched(arch):
        t = orig(arch)
        keep = "natural_log_exp_and_others"
        if keep not in t:
            return t
        return {k: (v if k == keep else set()) for k, v in t.items()}

    bacc.get_activation_tables = patched
    bass_interp.get_activation_tables = patched
    bacc._act_table_patched = True


def _plan(sa_sorted):
    """Slot plan from the sorted attribute vector.

    Returns (widths, per_core) where widths[p] is the (even) window width of
    position p and per_core[c][p] is (a_lo, a_hi, g0, g1, c0, L, wfac) or
    None for a dummy slot.
    """
    n = len(sa_sorted)
    bounds = [0]
    for i in range(1, n):
        if sa_sorted[i] != sa_sorted[i - 1]:
            bounds.append(i)
    bounds.append(n)

    # atoms: divisible window ranges tied to one job's anchors
    atoms = []  # (a_lo, a_hi, g0, g1, c_lo, c_hi, wfac)
    for gi in range(len(bounds) - 1):
        g0, g1 = bounds[gi], bounds[gi + 1]
        a = g0
        while a < g1:
            ah = min(a + 128, g1)
            atoms.append((a, ah, g0, g1, a, ah, 1.0))  # square (covers diag)
            if ah < g1:
                atoms.append((a, ah, g0, g1, ah, g1, 2.0))  # doubled tail
            a = ah

    def cut(W):
        pieces = []
        for a_lo, a_hi, g0, g1, c_lo, c_hi, wf in atoms:
            c = c_lo
            while c < c_hi:
                L = min(W, c_hi - c)
                pieces.append((L, (a_lo, a_hi, g0, g1, c, L, wf)))
                c += L
        pieces.sort(key=lambda x: -x[0])
        while len(pieces) % N_CORES:
            pieces.append((0, None))
        widths = []
        for p in range(0, len(pieces), N_CORES):
            w = max(x[0] for x in pieces[p : p + N_CORES])
            widths.append((w + 1) & ~1)  # even for bf16 4x alignment
        return widths, pieces

    best = None
    for W in range(100, 444, 4):
        widths, _ = cut(W)
        percore = sum(widths)
        npos = len(widths)
        # engine cost estimates (ns/col, ns/pos): ACT / DVE / PE-matmul
        est = max(
            27.5 * percore + 1100 * npos,
            8.3 * percore + 1980 * npos,
            13.3 * percore + 2150 * npos,
        )
        if best is None or est < best[0]:
            best = (est, W)
    widths, pieces = cut(best[1])

    per_core = [[] for _ in range(N_CORES)]
    for idx, (_, slot) in enumerate(pieces):
        per_core[idx % N_CORES].append(slot)
    return widths, per_core


def _build_program(widths):
    # Bacc (not raw Bass): its compile() runs generate_event_semaphores,
    # which splits multi-semaphore waits to satisfy the TRN2 one-wait-per-
    # instruction constraint this walrus build enforces.
    nc = bacc.Bacc(
        "TRN2", target_bir_lowering=False, debug=False, num_devices=N_CORES
    )
    f32 = mybir.dt.float32
    bf16 = mybir.dt.bfloat16
    S = len(widths)
    CW = sum(widths)
    W0 = widths[0]

    rep4_d = nc.dram_tensor("rep4", [128, CW], bf16, kind="ExternalInput").ap()
    scal_d = nc.dram_tensor("scal", [128, S * 32], f32, kind="ExternalInput").ap()
    lhsw_d = nc.dram_tensor("lhsw", [32, S * 128], bf16, kind="ExternalInput").ap()
    wsum_d = nc.dram_tensor("wsum", [32, S], bf16, kind="ExternalInput").ap()
    ndw_d = nc.dram_tensor("ndw", [128, S], f32, kind="ExternalInput").ap()
    acc0_d = nc.dram_tensor("acc0", [128, 1], f32, kind="ExternalInput").ap()
    ones_d = nc.dram_tensor("onesbd", [128, 8 * 32], bf16, kind="ExternalInput").ap()
    out_d = nc.dram_tensor("out", [128, 1], f32, kind="ExternalOutput").ap()

    Exp = mybir.ActivationFunctionType.Exp
    Ln = mybir.ActivationFunctionType.Ln
    mult = mybir.AluOpType.mult
    add = mybir.AluOpType.add

    with tile.TileContext(nc) as tc:
        with (
            tc.tile_pool(name="const", bufs=1) as cpool,
            tc.tile_pool(name="work", bufs=2) as wpool,
            tc.tile_pool(name="red", bufs=3) as rpool,
            tc.tile_pool(name="psE", bufs=2, space="PSUM") as psE,
            tc.tile_pool(name="psS", bufs=1, space="PSUM") as psS,
        ):
            rep4 = cpool.tile([128, CW], bf16, tag="rep4")
            nc.sync.dma_start(rep4[:], rep4_d[:])
            scal = cpool.tile([128, S * 32], f32, tag="scal")
            nc.sync.dma_start(scal[:], scal_d[:])
            lhsw = cpool.tile([32, S * 128], bf16, tag="lhsw")
            nc.sync.dma_start(lhsw[:], lhsw_d[:])
            wsum = cpool.tile([32, S], bf16, tag="wsum")
            nc.sync.dma_start(wsum[:], wsum_d[:])
            ndw = cpool.tile([128, S], f32, tag="ndw")
            nc.sync.dma_start(ndw[:], ndw_d[:])
            acc = cpool.tile([128, 1], f32, tag="acc")
            nc.sync.dma_start(acc[:], acc0_d[:])
            onesbd = cpool.tile([128, 8 * 32], bf16, tag="onesbd")
            nc.sync.dma_start(onesbd[:], ones_d[:])

            sSw = psS.tile([128, 1], f32, tag="sSw")

            off = 0
            for s, W in enumerate(widths):
                prod = wpool.tile([128, 32 * W0], bf16, tag="prod")
                for k in range(32):
                    nc.vector.tensor_scalar(
                        prod[:, k * W : (k + 1) * W],
                        rep4[:, off : off + W],
                        scal[:, s * 32 + k : s * 32 + k + 1],
                        None,
                        op0=mult,
                    )
                expt = wpool.tile([128, 32 * W0], bf16, tag="expt")
                nc.scalar.activation(expt[:, : 32 * W], prod[:, : 32 * W], Exp)

                nc.tensor.matmul(
                    sSw[:],
                    lhsT=lhsw[:, s * 128 : (s + 1) * 128],
                    rhs=wsum[:, s : s + 1],
                    start=(s == 0),
                    stop=(s == S - 1),
                )

                E = psE.tile([128, W0], f32, tag="E")
                for k in range(32):
                    h, i = divmod(k, 8)
                    nc.tensor.matmul(
                        E[32 * h : 32 * h + 32, :W],
                        lhsT=onesbd[:, 32 * i : 32 * (i + 1)],
                        rhs=expt[:, k * W : (k + 1) * W],
                        start=(i == 0),
                        stop=(i == 7),
                        tile_position=(0, 32 * h),
                    )

                lnout = rpool.tile([128, W0], bf16, tag="lnout")
                sL = rpool.tile([128, 1], f32, tag="sL")
                nc.scalar.activation(
                    lnout[:, :W], E[:, :W], Ln, accum_out=sL[:]
                )
                nc.vector.scalar_tensor_tensor(
                    acc[:], sL[:], ndw[:, s : s + 1], acc[:], op0=mult, op1=add
                )
                off += W

            res = rpool.tile([128, 1], f32, tag="res")
            nc.vector.tensor_add(res[:], acc[:], sSw[:])
            nc.sync.dma_start(out_d[:], res[:])

    nc.compile()
    return nc


def kernel(points, sensitive_attribute, t):
    _install_ntff_hook()
    _install_act_table_patch()

    points = np.asarray(points, dtype=np.float32)
    sa = np.asarray(sensitive_attribute).astype(np.int64)
    n, d = points.shape
    assert d == D

    scale = 1.0 / math.sqrt(float(np.asarray(t)))
    order = np.argsort(sa, kind="stable")
    sa_sorted = sa[order]
    ps = (points[order] * np.float32(scale)).astype(np.float32)  # [n, 32] sorted

    widths, per_core = _plan(sa_sorted)
    S = len(widths)
    CW = sum(widths)
    offs = np.concatenate([[0], np.cumsum(widths)]).astype(int)

    lnD = math.log(float(D))
    onesbd = np.zeros((128, 8 * 32), ml_dtypes.bfloat16)
    for i in range(8):
        for a in range(4):
            onesbd[32 * a : 32 * (a + 1), 36 * i + a] = 1.0

    in_maps = []
    for c in range(N_CORES):
        rep4 = np.zeros((128, CW), ml_dtypes.bfloat16)
        scal = np.zeros((128, S * 32), np.float32)
        lhsw = np.zeros((32, S * 128), ml_dtypes.bfloat16)
        wsum = np.zeros((32, S), ml_dtypes.bfloat16)
        ndw = np.zeros((128, S), np.float32)
        acc0 = np.zeros((128, 1), np.float32)
        for s, slot in enumerate(per_core[c]):
            if slot is None:
                continue  # dummy: zeros -> E=D, ln finite, weight 0
            a_lo, a_hi, g0, g1, c0, L, wfac = slot
            W = widths[s]
            m = a_hi - a_lo
            P = g1 - g0
            w = -wfac / (n * float(P) * float(P))
            win = ps[c0 : c0 + L].T  # [32, L]
            rep4[:, offs[s] : offs[s] + L] = np.tile(win, (4, 1))
            ablk = np.zeros((32, 128), np.float32)
            ablk[:, :m] = ps[a_lo:a_hi].T
            # scal col k = anchors 4k..4k+3 flattened (a-major, d-minor)
            scal[:, s * 32 : (s + 1) * 32] = ablk.T.reshape(32, 128).T
            lhsw[:, s * 128 : s * 128 + m] = np.float32(w) * ablk[:, :m]
            wsum[:, s] = win.sum(axis=1)
            ndw[:m, s] = -float(D) * w
            acc0[:m, 0] += w * D * lnD * (W - L)
        in_maps.append(
            {
                "rep4": rep4,
                "scal": scal,
                "lhsw": lhsw,
                "wsum": wsum,
                "ndw": ndw,
                "acc0": acc0,
                "onesbd": onesbd,
            }
        )

    nc = _build_program(widths)
    trace = bool(int(os.environ.get("KERNEL_TRACE", "0")))
    res = run_bass_kernel_spmd(nc, in_maps, list(range(N_CORES)), trace=trace)
    last_run_info["exec_time_ns"] = res.exec_time_ns
    last_run_info["mean_exec_time_ns"] = res.mean_exec_time_ns
    last_run_info["W"] = widths
    last_run_info["ntiles"] = S
    last_run_info["instructions"] = (
        res.instructions_and_trace[0] if res.instructions_and_trace else None
    )

    total = 0.0
    for c in range(N_CORES):
        total += float(res.results[c]["out"].astype(np.float64).sum())
    return np.float32(total)


# revision 11
# speedup vs baseline: 2.1643x; 1.0661x over previous
"""Trainium2 Bass kernel for the grouped contrastive loss.

Math: for anchors i and positives j in the same sensitive-attribute group g
(size P), with x_ij_d = p_i[d] p_j[d] / t:
    row(i,j) = S_ij - D * ln E_ij,   S_ij = sum_d x_ij_d,  E_ij = sum_d e^{x_ij_d}
(the log-softmax max-shift cancels analytically), and
    loss = sum_g -1/(N P^2) * sum_{i,j in g} row(i,j).

row is symmetric in (i,j), so after sorting points by group each group is
covered by per-block triangles: for each block B of <=128 consecutive sorted
anchors, process the full B x B square once (weight 1) plus the window
[B_end, g_end) (weight 2).  This does ~55% of the full-square element work
with plain per-row weights (the square covers both triangles + diagonal of
B x B exactly).

Work unit = slot: (job's <=128 anchors, a window piece of <=W cols).  Per
slot, on device (anchors packed 4-per-32-partition-span, dims on partitions):
  - 32 DVE tensor_scalar muls (bf16 in/out, 4x mode) build x for all packs.
  - one ACT Exp over [128, 32W] (bf16).
  - 32 PE matmuls vs a block-diagonal ones matrix accumulate each anchor's
    32 exp rows into its E row in PSUM ([128, W], quadrant cascades).
  - one ACT Ln over [128, W] with accum_out gives sum_j ln E per anchor.
  - sum_j S_ij = <a_i, sum_j w_j> collapses to one tiny PE matmul per slot
    (window column-sums precomputed host-side), weight folded into the
    anchor matrix, accumulated across slots in PSUM.
  - one gpsimd scalar_tensor_tensor folds -D * w * sumlnE into the f32
    accumulator.
Padding columns (zeros) contribute exactly D*ln(D) each to the ln-sum and 0
to the S-sum; the host folds the exact correction into the accumulator init.
All 8 cores run one SPMD program (identical slot-width schedule, per-core
data); each returns a [128] partial that the host sums.
"""

import math
import os
import sys

sys.path.insert(0, "/opt/trn_rl_repo")

import numpy as np
import ml_dtypes

import concourse.bacc as bacc
import concourse.bass as bass
import concourse.tile as tile
from concourse import mybir
from concourse.bass_utils import run_bass_kernel_spmd

N_CORES = 8
D = 32

last_run_info = {}


def _install_ntff_hook():
    # bass_utils' trace path under axon imports antenv.axon_hooks, which is
    # absent in this image; provide the ctypes-based hook it expects.
    import contextlib
    import ctypes
    import types

    if "antenv.axon_hooks" in sys.modules:
        return

    def _make_hook():
        try:
            lib = ctypes.CDLL("/opt/axon/libaxon_pjrt.so")
        except OSError:
            return None
        if not hasattr(lib, "axon_start_nrt_profile"):
            return None
        lib.axon_start_nrt_profile.argtypes = [
            ctypes.POINTER(ctypes.c_int64),
            ctypes.c_size_t,
        ]
        lib.axon_start_nrt_profile.restype = ctypes.c_int64
        lib.axon_stop_nrt_profile.argtypes = [ctypes.c_char_p]
        lib.axon_stop_nrt_profile.restype = ctypes.c_int64

        @contextlib.contextmanager
        def _hook_cm(output_dir, device_ids):
            import jax

            jax.devices()
            if device_ids:
                ids = (ctypes.c_int64 * len(device_ids))(*device_ids)
                rc = lib.axon_start_nrt_profile(ids, len(device_ids))
            else:
                rc = lib.axon_start_nrt_profile(None, 0)
            if rc != 0:
                raise RuntimeError(f"axon_start_nrt_profile rc={rc}")
            try:
                yield
            finally:
                n = lib.axon_stop_nrt_profile(str(output_dir).encode())
                if n < 0:
                    raise RuntimeError(f"axon_stop_nrt_profile rc={n}")

        return _hook_cm

    hook = _make_hook()
    mod = types.ModuleType("antenv.axon_hooks")
    mod.get_axon_ntff_profile_hook = lambda: hook
    mod.set_axon_ntff_profile_hook = lambda h: None
    sys.modules["antenv.axon_hooks"] = mod


def _install_act_table_patch():
    # The greedy act-table fixpoint picks the first table containing each
    # activation func, so Exp->exp_and_others and Ln->natural_log thrash
    # ACT_TABLE_LOADs (1283ns each) every tile. Mask every set except the
    # combined natural_log_exp_and_others (keeping dict order, hence the
    # act_func_set_id indices, intact) so one table serves both and the
    # load hoists out of the loop.
    import concourse.hw_specs as hw_specs
    import concourse.bass_interp as bass_interp

    if getattr(bacc, "_act_table_patched", False):
        return
    orig = hw_specs.get_activation_tables

    def patched(arch):
        t = orig(arch)
        keep = "natural_log_exp_and_others"
        if keep not in t:
            return t
        return {k: (v if k == keep else set()) for k, v in t.items()}

    bacc.get_activation_tables = patched
    bass_interp.get_activation_tables = patched
    bacc._act_table_patched = True


def _plan(sa_sorted):
    """Slot plan from the sorted attribute vector.

    Returns (widths, per_core) where widths[p] is the (even) window width of
    position p and per_core[c][p] is (a_lo, a_hi, g0, g1, c0, L, wfac) or
    None for a dummy slot.
    """
    n = len(sa_sorted)
    bounds = [0]
    for i in range(1, n):
        if sa_sorted[i] != sa_sorted[i - 1]:
            bounds.append(i)
    bounds.append(n)

    # atoms: divisible window ranges tied to one job's anchors
    atoms = []  # (a_lo, a_hi, g0, g1, c_lo, c_hi, wfac)
    for gi in range(len(bounds) - 1):
        g0, g1 = bounds[gi], bounds[gi + 1]
        a = g0
        while a < g1:
            ah = min(a + 128, g1)
            atoms.append((a, ah, g0, g1, a, ah, 1.0))  # square (covers diag)
            if ah < g1:
                atoms.append((a, ah, g0, g1, ah, g1, 2.0))  # doubled tail
            a = ah

    def cut(W):
        pieces = []
        for a_lo, a_hi, g0, g1, c_lo, c_hi, wf in atoms:
            c = c_lo
            while c < c_hi:
                L = min(W, c_hi - c)
                pieces.append((L, (a_lo, a_hi, g0, g1, c, L, wf)))
                c += L
        pieces.sort(key=lambda x: -x[0])
        while len(pieces) % N_CORES:
            pieces.append((0, None))
        widths = []
        for p in range(0, len(pieces), N_CORES):
            w = max(x[0] for x in pieces[p : p + N_CORES])
            widths.append((w + 1) & ~1)  # even for bf16 4x alignment
        return widths, pieces

    best = None
    for W in range(100, 444, 4):
        widths, _ = cut(W)
        percore = sum(widths)
        npos = len(widths)
        # engine cost estimates (ns/col, ns/pos): ACT / DVE / PE-matmul
        est = max(
            27.5 * percore + 1100 * npos,
            8.3 * percore + 1980 * npos,
            13.3 * percore + 2150 * npos,
        )
        if best is None or est < best[0]:
            best = (est, W)
    widths, pieces = cut(best[1])

    per_core = [[] for _ in range(N_CORES)]
    for idx, (_, slot) in enumerate(pieces):
        per_core[idx % N_CORES].append(slot)
    return widths, per_core


def _build_program(widths):
    # Bacc (not raw Bass): its compile() runs generate_event_semaphores,
    # which splits multi-semaphore waits to satisfy the TRN2 one-wait-per-
    # instruction constraint this walrus build enforces.
    nc = bacc.Bacc(
        "TRN2", target_bir_lowering=False, debug=False, num_devices=N_CORES
    )
    f32 = mybir.dt.float32
    bf16 = mybir.dt.bfloat16
    S = len(widths)
    CW = sum(widths)
    W0 = widths[0]

    rep4_d = nc.dram_tensor("rep4", [128, CW], bf16, kind="ExternalInput").ap()
    scal_d = nc.dram_tensor("scal", [128, S * 32], f32, kind="ExternalInput").ap()
    lhsw_d = nc.dram_tensor("lhsw", [32, S * 128], bf16, kind="ExternalInput").ap()
    wsum_d = nc.dram_tensor("wsum", [32, S], bf16, kind="ExternalInput").ap()
    ndw_d = nc.dram_tensor("ndw", [128, S], f32, kind="ExternalInput").ap()
    acc0_d = nc.dram_tensor("acc0", [128, 1], f32, kind="ExternalInput").ap()
    ones_d = nc.dram_tensor("onesbd", [128, 8 * 32], bf16, kind="ExternalInput").ap()
    out_d = nc.dram_tensor("out", [128, 1], f32, kind="ExternalOutput").ap()

    Exp = mybir.ActivationFunctionType.Exp
    Ln = mybir.ActivationFunctionType.Ln
    mult = mybir.AluOpType.mult
    add = mybir.AluOpType.add

    with tile.TileContext(nc) as tc:
        with (
            tc.tile_pool(name="const", bufs=1) as cpool,
            tc.tile_pool(name="work", bufs=3) as wpool,
            tc.tile_pool(name="red", bufs=3) as rpool,
            tc.tile_pool(name="psE", bufs=2, space="PSUM") as psE,
            tc.tile_pool(name="psS", bufs=1, space="PSUM") as psS,
        ):
            # Split big inputs per-slot across the three DMA paths (SP/ACT
            # HWDGE + Pool SWDGE) so slot 0's data lands in ~3us instead of
            # waiting on one serialized ~11us queue.
            scal_t = []
            rep4_t = []
            offp = 0
            for s, W in enumerate(widths):
                st = cpool.tile([128, 32], f32, tag=f"scal{s}")
                nc.sync.dma_start(st[:], scal_d[:, s * 32 : (s + 1) * 32])
                rt = cpool.tile([128, W], bf16, tag=f"rep4{s}")
                nc.gpsimd.dma_start(rt[:], rep4_d[:, offp : offp + W])
                scal_t.append(st)
                rep4_t.append(rt)
                offp += W
            lhsw = cpool.tile([32, S * 128], bf16, tag="lhsw")
            nc.scalar.dma_start(lhsw[:], lhsw_d[:])
            onesbd = cpool.tile([128, 8 * 32], bf16, tag="onesbd")
            nc.scalar.dma_start(onesbd[:], ones_d[:])
            wsum = cpool.tile([32, S], bf16, tag="wsum")
            nc.sync.dma_start(wsum[:], wsum_d[:])
            ndw = cpool.tile([128, S], f32, tag="ndw")
            nc.sync.dma_start(ndw[:], ndw_d[:])
            acc = cpool.tile([128, 1], f32, tag="acc")
            nc.sync.dma_start(acc[:], acc0_d[:])

            sSw = psS.tile([128, 1], f32, tag="sSw")

            for s, W in enumerate(widths):
                prod = wpool.tile([128, 32 * W0], bf16, tag="prod")
                for k in range(32):
                    nc.vector.tensor_scalar(
                        prod[:, k * W : (k + 1) * W],
                        rep4_t[s][:],
                        scal_t[s][:, k : k + 1],
                        None,
                        op0=mult,
                    )
                expt = wpool.tile([128, 32 * W0], bf16, tag="expt")
                nc.scalar.activation(expt[:, : 32 * W], prod[:, : 32 * W], Exp)

                nc.tensor.matmul(
                    sSw[:],
                    lhsT=lhsw[:, s * 128 : (s + 1) * 128],
                    rhs=wsum[:, s : s + 1],
                    start=(s == 0),
                    stop=(s == S - 1),
                )

                E = psE.tile([128, W0], f32, tag="E")
                for k in range(32):
                    h, i = divmod(k, 8)
                    nc.tensor.matmul(
                        E[32 * h : 32 * h + 32, :W],
                        lhsT=onesbd[:, 32 * i : 32 * (i + 1)],
                        rhs=expt[:, k * W : (k + 1) * W],
                        start=(i == 0),
                        stop=(i == 7),
                        tile_position=(0, 32 * h),
                    )

                lnout = rpool.tile([128, W0], bf16, tag="lnout")
                sL = rpool.tile([128, 1], f32, tag="sL")
                nc.scalar.activation(
                    lnout[:, :W], E[:, :W], Ln, accum_out=sL[:]
                )
                nc.vector.scalar_tensor_tensor(
                    acc[:], sL[:], ndw[:, s : s + 1], acc[:], op0=mult, op1=add
                )

            res = rpool.tile([128, 1], f32, tag="res")
            nc.vector.tensor_add(res[:], acc[:], sSw[:])
            nc.sync.dma_start(out_d[:], res[:])

    nc.compile()
    return nc


def kernel(points, sensitive_attribute, t):
    _install_ntff_hook()
    _install_act_table_patch()

    points = np.asarray(points, dtype=np.float32)
    sa = np.asarray(sensitive_attribute).astype(np.int64)
    n, d = points.shape
    assert d == D

    scale = 1.0 / math.sqrt(float(np.asarray(t)))
    order = np.argsort(sa, kind="stable")
    sa_sorted = sa[order]
    ps = (points[order] * np.float32(scale)).astype(np.float32)  # [n, 32] sorted

    widths, per_core = _plan(sa_sorted)
    S = len(widths)
    CW = sum(widths)
    offs = np.concatenate([[0], np.cumsum(widths)]).astype(int)

    lnD = math.log(float(D))
    onesbd = np.zeros((128, 8 * 32), ml_dtypes.bfloat16)
    for i in range(8):
        for a in range(4):
            onesbd[32 * a : 32 * (a + 1), 36 * i + a] = 1.0

    in_maps = []
    for c in range(N_CORES):
        rep4 = np.zeros((128, CW), ml_dtypes.bfloat16)
        scal = np.zeros((128, S * 32), np.float32)
        lhsw = np.zeros((32, S * 128), ml_dtypes.bfloat16)
        wsum = np.zeros((32, S), ml_dtypes.bfloat16)
        ndw = np.zeros((128, S), np.float32)
        acc0 = np.zeros((128, 1), np.float32)
        for s, slot in enumerate(per_core[c]):
            if slot is None:
                continue  # dummy: zeros -> E=D, ln finite, weight 0
            a_lo, a_hi, g0, g1, c0, L, wfac = slot
            W = widths[s]
            m = a_hi - a_lo
            P = g1 - g0
            w = -wfac / (n * float(P) * float(P))
            win = ps[c0 : c0 + L].T  # [32, L]
            rep4[:, offs[s] : offs[s] + L] = np.tile(win, (4, 1))
            ablk = np.zeros((32, 128), np.float32)
            ablk[:, :m] = ps[a_lo:a_hi].T
            # scal col k = anchors 4k..4k+3 flattened (a-major, d-minor)
            scal[:, s * 32 : (s + 1) * 32] = ablk.T.reshape(32, 128).T
            lhsw[:, s * 128 : s * 128 + m] = np.float32(w) * ablk[:, :m]
            wsum[:, s] = win.sum(axis=1)
            ndw[:m, s] = -float(D) * w
            acc0[:m, 0] += w * D * lnD * (W - L)
        in_maps.append(
            {
                "rep4": rep4,
                "scal": scal,
                "lhsw": lhsw,
                "wsum": wsum,
                "ndw": ndw,
                "acc0": acc0,
                "onesbd": onesbd,
            }
        )

    nc = _build_program(widths)
    trace = bool(int(os.environ.get("KERNEL_TRACE", "0")))
    res = run_bass_kernel_spmd(nc, in_maps, list(range(N_CORES)), trace=trace)
    last_run_info["exec_time_ns"] = res.exec_time_ns
    last_run_info["mean_exec_time_ns"] = res.mean_exec_time_ns
    last_run_info["W"] = widths
    last_run_info["ntiles"] = S
    last_run_info["instructions"] = (
        res.instructions_and_trace[0] if res.instructions_and_trace else None
    )

    total = 0.0
    for c in range(N_CORES):
        total += float(res.results[c]["out"].astype(np.float64).sum())
    return np.float32(total)


# revision 26
# speedup vs baseline: 2.1815x; 1.0080x over previous
"""Trainium2 Bass kernel for the grouped contrastive loss.

Math: for anchors i and positives j in the same sensitive-attribute group g
(size P), with x_ij_d = p_i[d] p_j[d] / t:
    row(i,j) = S_ij - D * ln E_ij,   S_ij = sum_d x_ij_d,  E_ij = sum_d e^{x_ij_d}
(the log-softmax max-shift cancels analytically), and
    loss = sum_g -1/(N P^2) * sum_{i,j in g} row(i,j).

row is symmetric in (i,j), so after sorting points by group each group is
covered by per-block triangles: for each block B of <=128 consecutive sorted
anchors, process the full B x B square once (weight 1) plus the window
[B_end, g_end) (weight 2).  This does ~55% of the full-square element work
with plain per-row weights (the square covers both triangles + diagonal of
B x B exactly).

Work unit = slot: (job's <=128 anchors, a window piece of <=W cols); slots
are grouped into 8-wide positions with a shared width so all 8 cores run one
SPMD program (identical slot schedule, per-core data).  Per slot, on device
(anchors packed 4-per-32-partition-span, dims on partitions):
  - 32 DVE tensor_scalar muls (bf16 in/out, 2x mode) build x for all packs.
  - one ACT Exp over [128, 32W] bf16, emitted in two halves so the PE
    cascade starts at half-time.
  - 32 PE matmuls vs a block-diagonal ones matrix accumulate each anchor's
    32 exp rows into its E row in PSUM ([128, W], quadrant cascades).
  - one ACT Ln over [128, W] whose accum_out writes sum_j ln E into this
    slot's column of a [128, S] matrix; one weighted row-reduce at the very
    end folds -D * w * sumlnE for all slots at once.
  - sum_j S_ij = <a_i, sum_j w_j> collapses to one tiny PE matmul per slot
    (window column-sums precomputed host-side), weight folded into the
    anchor matrix, accumulated across slots in PSUM.
Padding columns (zeros) contribute exactly D*ln(D) each to the ln-sum and 0
to the S-sum; the host folds the exact correction into the accumulator init.
Each core returns a [128] partial that the host sums.
"""

import math
import os
import sys

sys.path.insert(0, "/opt/trn_rl_repo")

import numpy as np
import ml_dtypes

import concourse.bacc as bacc
import concourse.tile as tile
from concourse import mybir
from concourse.bass_utils import run_bass_kernel_spmd

N_CORES = 8
D = 32

last_run_info = {}
_nc_cache = {}


def _install_ntff_hook():
    # bass_utils' trace path under axon imports antenv.axon_hooks, which is
    # absent in this image; provide the ctypes-based hook it expects.
    import contextlib
    import ctypes
    import types

    if "antenv.axon_hooks" in sys.modules:
        return

    def _make_hook():
        try:
            lib = ctypes.CDLL("/opt/axon/libaxon_pjrt.so")
        except OSError:
            return None
        if not hasattr(lib, "axon_start_nrt_profile"):
            return None
        lib.axon_start_nrt_profile.argtypes = [
            ctypes.POINTER(ctypes.c_int64),
            ctypes.c_size_t,
        ]
        lib.axon_start_nrt_profile.restype = ctypes.c_int64
        lib.axon_stop_nrt_profile.argtypes = [ctypes.c_char_p]
        lib.axon_stop_nrt_profile.restype = ctypes.c_int64

        @contextlib.contextmanager
        def _hook_cm(output_dir, device_ids):
            import jax

            jax.devices()
            if device_ids:
                ids = (ctypes.c_int64 * len(device_ids))(*device_ids)
                rc = lib.axon_start_nrt_profile(ids, len(device_ids))
            else:
                rc = lib.axon_start_nrt_profile(None, 0)
            if rc != 0:
                raise RuntimeError(f"axon_start_nrt_profile rc={rc}")
            try:
                yield
            finally:
                n = lib.axon_stop_nrt_profile(str(output_dir).encode())
                if n < 0:
                    raise RuntimeError(f"axon_stop_nrt_profile rc={n}")

        return _hook_cm

    hook = _make_hook()
    mod = types.ModuleType("antenv.axon_hooks")
    mod.get_axon_ntff_profile_hook = lambda: hook
    mod.set_axon_ntff_profile_hook = lambda h: None
    sys.modules["antenv.axon_hooks"] = mod


def _install_act_table_patch():
    # The greedy act-table fixpoint picks the first table containing each
    # activation func, so Exp->exp_and_others and Ln->natural_log thrash
    # ACT_TABLE_LOADs (1283ns each) every tile. Mask every set except the
    # combined natural_log_exp_and_others (keeping dict order, hence the
    # act_func_set_id indices, intact) so one table serves both and the
    # load hoists out of the loop.
    import concourse.hw_specs as hw_specs
    import concourse.bass_interp as bass_interp

    if getattr(bacc, "_act_table_patched", False):
        return
    orig = hw_specs.get_activation_tables

    def patched(arch):
        t = orig(arch)
        keep = "natural_log_exp_and_others"
        if keep not in t:
            return t
        return {k: (v if k == keep else set()) for k, v in t.items()}

    bacc.get_activation_tables = patched
    bass_interp.get_activation_tables = patched
    bacc._act_table_patched = True


def _plan(sa_sorted):
    """Slot plan from the sorted attribute vector.

    Returns (widths, per_core) where widths[p] is the (even) window width of
    position p and per_core[c][p] is (a_lo, a_hi, g0, g1, c0, L, wfac) or
    None for a dummy slot.
    """
    n = len(sa_sorted)
    bounds = [0]
    for i in range(1, n):
        if sa_sorted[i] != sa_sorted[i - 1]:
            bounds.append(i)
    bounds.append(n)

    # atoms: divisible window ranges tied to one job's anchors
    atoms = []  # (a_lo, a_hi, g0, g1, c_lo, c_hi, wfac)
    for gi in range(len(bounds) - 1):
        g0, g1 = bounds[gi], bounds[gi + 1]
        a = g0
        while a < g1:
            ah = min(a + 128, g1)
            atoms.append((a, ah, g0, g1, a, ah, 1.0))  # square (covers diag)
            if ah < g1:
                atoms.append((a, ah, g0, g1, ah, g1, 2.0))  # doubled tail
            a = ah

    def cut(W):
        pieces = []
        for a_lo, a_hi, g0, g1, c_lo, c_hi, wf in atoms:
            c = c_lo
            while c < c_hi:
                L = min(W, c_hi - c)
                pieces.append((L, (a_lo, a_hi, g0, g1, c, L, wf)))
                c += L
        pieces.sort(key=lambda x: -x[0])
        while len(pieces) % N_CORES:
            pieces.append((0, None))
        widths = []
        for p in range(0, len(pieces), N_CORES):
            w = max(x[0] for x in pieces[p : p + N_CORES])
            widths.append((w + 1) & ~1)  # even for bf16 4x alignment
        return widths, pieces

    best = None
    for W in range(100, 444, 4):
        widths, _ = cut(W)
        percore = sum(widths)
        npos = len(widths)
        # engine cost estimates (ns/col, ns/pos): ACT / DVE / PE-matmul
        est = max(
            27.5 * percore + 1100 * npos,
            8.3 * percore + 1980 * npos,
            13.3 * percore + 2150 * npos,
        )
        if best is None or est < best[0]:
            best = (est, W)
    widths, pieces = cut(best[1])

    per_core = [[] for _ in range(N_CORES)]
    for idx, (_, slot) in enumerate(pieces):
        per_core[idx % N_CORES].append(slot)
    # Swap the two widest positions: the first slot's window chunk gates the
    # whole pipeline start, so lead with the second-widest (smaller DMA).
    if len(widths) > 1:
        widths[0], widths[1] = widths[1], widths[0]
        for slots in per_core:
            slots[0], slots[1] = slots[1], slots[0]
    return widths, per_core


def _build_program(widths):
    # Bacc (not raw Bass): its compile() runs generate_event_semaphores,
    # which splits multi-semaphore waits to satisfy the TRN2 one-wait-per-
    # instruction constraint this walrus build enforces.
    nc = bacc.Bacc(
        "TRN2", target_bir_lowering=False, debug=False, num_devices=N_CORES
    )
    f32 = mybir.dt.float32
    bf16 = mybir.dt.bfloat16
    S = len(widths)
    CW = sum(widths)
    W0 = max(widths)

    rep4_d = nc.dram_tensor("rep4", [128, CW], bf16, kind="ExternalInput").ap()
    scal_d = nc.dram_tensor("scal", [128, S * 32], f32, kind="ExternalInput").ap()
    lhsw_d = nc.dram_tensor("lhsw", [32, S * 128], bf16, kind="ExternalInput").ap()
    wsum_d = nc.dram_tensor("wsum", [32, S], bf16, kind="ExternalInput").ap()
    ndw_d = nc.dram_tensor("ndw", [128, S], f32, kind="ExternalInput").ap()
    acc0_d = nc.dram_tensor("acc0", [128, 1], f32, kind="ExternalInput").ap()
    ones_d = nc.dram_tensor("onesbd", [128, 8 * 32], bf16, kind="ExternalInput").ap()
    out_d = nc.dram_tensor("out", [128, 1], f32, kind="ExternalOutput").ap()

    Exp = mybir.ActivationFunctionType.Exp
    Ln = mybir.ActivationFunctionType.Ln
    mult = mybir.AluOpType.mult
    add = mybir.AluOpType.add

    with tile.TileContext(nc) as tc:
        with (
            tc.tile_pool(name="const", bufs=1) as cpool,
            tc.tile_pool(name="work", bufs=3) as wpool,
            tc.tile_pool(name="red", bufs=3) as rpool,
            tc.tile_pool(name="psE", bufs=2, space="PSUM") as psE,
            tc.tile_pool(name="psS", bufs=1, space="PSUM") as psS,
        ):
            # Split big inputs per-slot across the three DMA paths (SP/ACT
            # HWDGE + Pool SWDGE) so slot 0's data lands in ~3us instead of
            # waiting on one serialized ~11us queue.
            scal_t = []
            rep4_t = []
            offs_d = [0]
            for W in widths:
                offs_d.append(offs_d[-1] + W)
            for s, W in enumerate(widths):
                scal_t.append(cpool.tile([128, 32], f32, tag=f"scal{s}", name=f"scal{s}"))
                rep4_t.append(cpool.tile([128, W], bf16, tag=f"rep4{s}", name=f"rep4{s}"))
            # Slot 0's data first, on the earliest-available queues (SP for
            # the big window chunk, ACT for the scalars); later chunks go to
            # the Pool SWDGE / remaining SP slots so transfers overlap.
            h0 = widths[0] // 2
            nc.sync.dma_start(rep4_t[0][:, :h0], rep4_d[:, 0:h0])
            nc.scalar.dma_start(scal_t[0][:], scal_d[:, 0:32])
            nc.sync.dma_start(
                rep4_t[0][:, h0 : widths[0]], rep4_d[:, h0 : widths[0]]
            )
            if S > 1:
                nc.sync.dma_start(
                    rep4_t[1][:], rep4_d[:, offs_d[1] : offs_d[1] + widths[1]]
                )
                nc.scalar.dma_start(scal_t[1][:], scal_d[:, 32:64])
            for s, W in enumerate(widths):
                if s < 2:
                    continue
                nc.sync.dma_start(scal_t[s][:], scal_d[:, s * 32 : (s + 1) * 32])
                nc.gpsimd.dma_start(rep4_t[s][:], rep4_d[:, offs_d[s] : offs_d[s] + W])
            lhsw = cpool.tile([32, S * 128], bf16, tag="lhsw")
            nc.scalar.dma_start(lhsw[:], lhsw_d[:])
            onesbd = cpool.tile([128, 8 * 32], bf16, tag="onesbd")
            nc.scalar.dma_start(onesbd[:], ones_d[:])
            wsum = cpool.tile([32, S], bf16, tag="wsum")
            nc.sync.dma_start(wsum[:], wsum_d[:])
            ndw = cpool.tile([128, S], f32, tag="ndw")
            nc.sync.dma_start(ndw[:], ndw_d[:])
            acc = cpool.tile([128, 1], f32, tag="acc")
            nc.sync.dma_start(acc[:], acc0_d[:])

            sSw = psS.tile([128, 1], f32, tag="sSw")
            sLm = cpool.tile([128, S], f32, tag="sLm")

            def emit_ln(E, W, s):
                lnout = rpool.tile([128, W0], bf16, tag="lnout")
                nc.scalar.activation(
                    lnout[:, :W], E[:, :W], Ln, accum_out=sLm[:, s : s + 1]
                )

            def emit_muls(s, W):
                prod = wpool.tile([128, 32 * W0], bf16, tag="prod", name="prod")
                for k in range(32):
                    nc.vector.tensor_scalar(
                        prod[:, k * W : (k + 1) * W],
                        rep4_t[s][:],
                        scal_t[s][:, k : k + 1],
                        None,
                        op0=mult,
                    )
                return prod

            # The last slot's muls are hoisted early in the DVE stream so
            # the final exp never stalls on the init-bound tail muls.
            prods = {}
            pend = None
            for s, W in enumerate(widths):
                if s not in prods:
                    prods[s] = emit_muls(s, W)
                prod = prods.pop(s)
                # exp in two halves so the PE cascade starts at half-time;
                # the previous slot's ln is emitted after this exp so its
                # matmuls get covered by the exp run.
                expt = wpool.tile([128, 32 * W0], bf16, tag="expt")
                nc.scalar.activation(
                    expt[:, : 16 * W], prod[:, : 16 * W], Exp
                )
                nc.scalar.activation(
                    expt[:, 16 * W : 32 * W], prod[:, 16 * W : 32 * W], Exp
                )
                if pend is not None:
                    emit_ln(*pend)

                nc.tensor.matmul(
                    sSw[:],
                    lhsT=lhsw[:, s * 128 : (s + 1) * 128],
                    rhs=wsum[:, s : s + 1],
                    start=(s == 0),
                    stop=(s == S - 1),
                )

                E = psE.tile([128, W0], f32, tag="E")
                for k in range(32):
                    h, i = divmod(k, 8)
                    nc.tensor.matmul(
                        E[32 * h : 32 * h + 32, :W],
                        lhsT=onesbd[:, 32 * i : 32 * (i + 1)],
                        rhs=expt[:, k * W : (k + 1) * W],
                        start=(i == 0),
                        stop=(i == 7),
                        tile_position=(0, 32 * h),
                    )
                pend = (E, W, s)
            emit_ln(*pend)

            # res = acc0 + sSw + sum_s ndw[:,s] * sLm[:,s]
            wl = rpool.tile([128, S], f32, tag="wl")
            nc.vector.tensor_tensor(wl[:], sLm[:], ndw[:], mult)
            red = rpool.tile([128, 1], f32, tag="red")
            nc.vector.tensor_reduce(
                red[:], wl[:], axis=mybir.AxisListType.X, op=add
            )
            base = rpool.tile([128, 1], f32, tag="base")
            nc.vector.tensor_add(base[:], acc[:], sSw[:])
            res = rpool.tile([128, 1], f32, tag="res")
            nc.vector.tensor_add(res[:], base[:], red[:])
            nc.gpsimd.dma_start(out_d[:], res[:])

    nc.compile()
    return nc


def kernel(points, sensitive_attribute, t):
    _install_ntff_hook()
    _install_act_table_patch()

    points = np.asarray(points, dtype=np.float32)
    sa = np.asarray(sensitive_attribute).astype(np.int64)
    n, d = points.shape
    assert d == D

    scale = 1.0 / math.sqrt(float(np.asarray(t)))
    order = np.argsort(sa, kind="stable")
    sa_sorted = sa[order]
    ps = (points[order] * np.float32(scale)).astype(np.float32)  # [n, 32] sorted

    widths, per_core = _plan(sa_sorted)
    S = len(widths)
    CW = sum(widths)
    offs = np.concatenate([[0], np.cumsum(widths)]).astype(int)

    lnD = math.log(float(D))
    onesbd = np.zeros((128, 8 * 32), ml_dtypes.bfloat16)
    for i in range(8):
        for a in range(4):
            onesbd[32 * a : 32 * (a + 1), 36 * i + a] = 1.0

    in_maps = []
    for c in range(N_CORES):
        rep4 = np.zeros((128, CW), ml_dtypes.bfloat16)
        scal = np.zeros((128, S * 32), np.float32)
        lhsw = np.zeros((32, S * 128), ml_dtypes.bfloat16)
        wsum = np.zeros((32, S), ml_dtypes.bfloat16)
        ndw = np.zeros((128, S), np.float32)
        acc0 = np.zeros((128, 1), np.float32)
        for s, slot in enumerate(per_core[c]):
            if slot is None:
                continue  # dummy: zeros -> E=D, ln finite, weight 0
            a_lo, a_hi, g0, g1, c0, L, wfac = slot
            W = widths[s]
            m = a_hi - a_lo
            P = g1 - g0
            w = -wfac / (n * float(P) * float(P))
            win = ps[c0 : c0 + L].T  # [32, L]
            rep4[:, offs[s] : offs[s] + L] = np.tile(win, (4, 1))
            ablk = np.zeros((32, 128), np.float32)
            ablk[:, :m] = ps[a_lo:a_hi].T
            # scal col k = anchors 4k..4k+3 flattened (a-major, d-minor)
            scal[:, s * 32 : (s + 1) * 32] = ablk.T.reshape(32, 128).T
            lhsw[:, s * 128 : s * 128 + m] = np.float32(w) * ablk[:, :m]
            wsum[:, s] = win.sum(axis=1)
            ndw[:m, s] = -float(D) * w
            acc0[:m, 0] += w * D * lnD * (W - L)
        in_maps.append(
            {
                "rep4": rep4,
                "scal": scal,
                "lhsw": lhsw,
                "wsum": wsum,
                "ndw": ndw,
                "acc0": acc0,
                "onesbd": onesbd,
            }
        )

    key = tuple(widths)
    nc = _nc_cache.get(key)
    if nc is None:
        nc = _build_program(widths)
        _nc_cache[key] = nc
    trace = bool(int(os.environ.get("KERNEL_TRACE", "0")))
    res = run_bass_kernel_spmd(nc, in_maps, list(range(N_CORES)), trace=trace)
    last_run_info["exec_time_ns"] = res.exec_time_ns
    last_run_info["mean_exec_time_ns"] = res.mean_exec_time_ns
    last_run_info["W"] = widths
    last_run_info["ntiles"] = S
    last_run_info["instructions"] = (
        res.instructions_and_trace[0] if res.instructions_and_trace else None
    )

    total = 0.0
    for c in range(N_CORES):
        total += float(res.results[c]["out"].astype(np.float64).sum())
    return np.float32(total)


# revision 29
# speedup vs baseline: 2.2931x; 1.0512x over previous
"""Trainium2 Bass kernel for the grouped contrastive loss.

Math: for anchors i and positives j in the same sensitive-attribute group g
(size P), with x_ij_d = p_i[d] p_j[d] / t:
    row(i,j) = S_ij - D * ln E_ij,   S_ij = sum_d x_ij_d,  E_ij = sum_d e^{x_ij_d}
(the log-softmax max-shift cancels analytically), and
    loss = sum_g -1/(N P^2) * sum_{i,j in g} row(i,j).

row is symmetric in (i,j), so after sorting points by group each group is
covered by per-block triangles: for each block B of <=128 consecutive sorted
anchors, process the full B x B square once (weight 1) plus the window
[B_end, g_end) (weight 2).  This does ~55% of the full-square element work
with plain per-row weights (the square covers both triangles + diagonal of
B x B exactly).

Work unit = slot: (job's <=128 anchors, a window piece of <=W cols); slots
are grouped into 8-wide positions with a shared width so all 8 cores run one
SPMD program (identical slot schedule, per-core data).  Per slot, on device
(anchors packed 4-per-32-partition-span, dims on partitions):
  - 32 DVE tensor_scalar muls (bf16 in/out, 2x mode) build x for all packs.
  - one ACT Exp over [128, 32W] bf16, emitted in two halves so the PE
    cascade starts at half-time.
  - 32 PE matmuls vs a block-diagonal ones matrix accumulate each anchor's
    32 exp rows into its E row in PSUM ([128, W], quadrant cascades).
  - one ACT Ln over [128, W] whose accum_out writes sum_j ln E into this
    slot's column of a [128, S] matrix; one weighted row-reduce at the very
    end folds -D * w * sumlnE for all slots at once.
  - sum_j S_ij = <a_i, sum_j w_j> collapses to one tiny PE matmul per slot
    (window column-sums precomputed host-side), weight folded into the
    anchor matrix, accumulated across slots in PSUM.
Padding columns (zeros) contribute exactly D*ln(D) each to the ln-sum and 0
to the S-sum; the host folds the exact correction into the accumulator init.
Each core returns a [128] partial that the host sums.
"""

import math
import os
import sys

sys.path.insert(0, "/opt/trn_rl_repo")

import numpy as np
import ml_dtypes

import concourse.bacc as bacc
import concourse.tile as tile
from concourse import mybir
from concourse.bass_utils import run_bass_kernel_spmd

N_CORES = 8
D = 32

last_run_info = {}
_nc_cache = {}


def _install_ntff_hook():
    # bass_utils' trace path under axon imports antenv.axon_hooks, which is
    # absent in this image; provide the ctypes-based hook it expects.
    import contextlib
    import ctypes
    import types

    if "antenv.axon_hooks" in sys.modules:
        return

    def _make_hook():
        try:
            lib = ctypes.CDLL("/opt/axon/libaxon_pjrt.so")
        except OSError:
            return None
        if not hasattr(lib, "axon_start_nrt_profile"):
            return None
        lib.axon_start_nrt_profile.argtypes = [
            ctypes.POINTER(ctypes.c_int64),
            ctypes.c_size_t,
        ]
        lib.axon_start_nrt_profile.restype = ctypes.c_int64
        lib.axon_stop_nrt_profile.argtypes = [ctypes.c_char_p]
        lib.axon_stop_nrt_profile.restype = ctypes.c_int64

        @contextlib.contextmanager
        def _hook_cm(output_dir, device_ids):
            import jax

            jax.devices()
            if device_ids:
                ids = (ctypes.c_int64 * len(device_ids))(*device_ids)
                rc = lib.axon_start_nrt_profile(ids, len(device_ids))
            else:
                rc = lib.axon_start_nrt_profile(None, 0)
            if rc != 0:
                raise RuntimeError(f"axon_start_nrt_profile rc={rc}")
            try:
                yield
            finally:
                n = lib.axon_stop_nrt_profile(str(output_dir).encode())
                if n < 0:
                    raise RuntimeError(f"axon_stop_nrt_profile rc={n}")

        return _hook_cm

    hook = _make_hook()
    mod = types.ModuleType("antenv.axon_hooks")
    mod.get_axon_ntff_profile_hook = lambda: hook
    mod.set_axon_ntff_profile_hook = lambda h: None
    sys.modules["antenv.axon_hooks"] = mod


def _install_act_table_patch():
    # The greedy act-table fixpoint picks the first table containing each
    # activation func, so Exp->exp_and_others and Ln->natural_log thrash
    # ACT_TABLE_LOADs (1283ns each) every tile. Mask every set except the
    # combined natural_log_exp_and_others (keeping dict order, hence the
    # act_func_set_id indices, intact) so one table serves both and the
    # load hoists out of the loop.
    import concourse.hw_specs as hw_specs
    import concourse.bass_interp as bass_interp

    if getattr(bacc, "_act_table_patched", False):
        return
    orig = hw_specs.get_activation_tables

    def patched(arch):
        t = orig(arch)
        keep = "natural_log_exp_and_others"
        if keep not in t:
            return t
        return {k: (v if k == keep else set()) for k, v in t.items()}

    bacc.get_activation_tables = patched
    bass_interp.get_activation_tables = patched
    bacc._act_table_patched = True


def _plan(sa_sorted):
    """Slot plan from the sorted attribute vector.

    Returns (widths, per_core) where widths[p] is the (even) window width of
    position p and per_core[c][p] is (a_lo, a_hi, g0, g1, c0, L, wfac) or
    None for a dummy slot.
    """
    n = len(sa_sorted)
    bounds = [0]
    for i in range(1, n):
        if sa_sorted[i] != sa_sorted[i - 1]:
            bounds.append(i)
    bounds.append(n)

    # atoms: divisible window ranges tied to one job's anchors
    atoms = []  # (a_lo, a_hi, g0, g1, c_lo, c_hi, wfac)
    for gi in range(len(bounds) - 1):
        g0, g1 = bounds[gi], bounds[gi + 1]
        a = g0
        while a < g1:
            ah = min(a + 128, g1)
            atoms.append((a, ah, g0, g1, a, ah, 1.0))  # square (covers diag)
            if ah < g1:
                atoms.append((a, ah, g0, g1, ah, g1, 2.0))  # doubled tail
            a = ah

    def cut(W):
        pieces = []
        for a_lo, a_hi, g0, g1, c_lo, c_hi, wf in atoms:
            c = c_lo
            while c < c_hi:
                L = min(W, c_hi - c)
                pieces.append((L, (a_lo, a_hi, g0, g1, c, L, wf)))
                c += L
        pieces.sort(key=lambda x: -x[0])
        while len(pieces) % N_CORES:
            pieces.append((0, None))
        widths = []
        for p in range(0, len(pieces), N_CORES):
            w = max(x[0] for x in pieces[p : p + N_CORES])
            widths.append((w + 1) & ~1)  # even for bf16 4x alignment
        return widths, pieces

    best = None
    for W in range(100, 444, 4):
        widths, _ = cut(W)
        percore = sum(widths)
        npos = len(widths)
        # engine cost estimates (ns/col, ns/pos): ACT / DVE / PE-matmul
        est = max(
            27.5 * percore + 1100 * npos,
            8.3 * percore + 1980 * npos,
            13.3 * percore + 2150 * npos,
        )
        if best is None or est < best[0]:
            best = (est, W)
    widths, pieces = cut(best[1])

    per_core = [[] for _ in range(N_CORES)]
    for idx, (_, slot) in enumerate(pieces):
        per_core[idx % N_CORES].append(slot)
    # Swap the two widest positions: the first slot's window chunk gates the
    # whole pipeline start, so lead with the second-widest (smaller DMA).
    if len(widths) > 1:
        widths[0], widths[1] = widths[1], widths[0]
        for slots in per_core:
            slots[0], slots[1] = slots[1], slots[0]
    return widths, per_core


def _build_program(widths):
    # Bacc (not raw Bass): its compile() runs generate_event_semaphores,
    # which splits multi-semaphore waits to satisfy the TRN2 one-wait-per-
    # instruction constraint this walrus build enforces.
    nc = bacc.Bacc(
        "TRN2", target_bir_lowering=False, debug=False, num_devices=N_CORES
    )
    f32 = mybir.dt.float32
    bf16 = mybir.dt.bfloat16
    S = len(widths)
    CW = sum(widths)
    W0 = max(widths)

    rep4_d = nc.dram_tensor("rep4", [128, CW], bf16, kind="ExternalInput").ap()
    scal_d = nc.dram_tensor("scal", [128, S * 32], f32, kind="ExternalInput").ap()
    lhsw_d = nc.dram_tensor("lhsw", [32, S * 128], bf16, kind="ExternalInput").ap()
    wsum_d = nc.dram_tensor("wsum", [32, S], bf16, kind="ExternalInput").ap()
    ndw_d = nc.dram_tensor("ndw", [128, S], f32, kind="ExternalInput").ap()
    acc0_d = nc.dram_tensor("acc0", [128, 1], f32, kind="ExternalInput").ap()
    ones_d = nc.dram_tensor("onesbd", [128, 8 * 32], bf16, kind="ExternalInput").ap()
    out_d = nc.dram_tensor("out", [128, 1], f32, kind="ExternalOutput").ap()

    Exp = mybir.ActivationFunctionType.Exp
    Ln = mybir.ActivationFunctionType.Ln
    mult = mybir.AluOpType.mult
    add = mybir.AluOpType.add

    with tile.TileContext(nc) as tc:
        with (
            tc.tile_pool(name="const", bufs=1) as cpool,
            tc.tile_pool(name="work", bufs=4) as wpool,
            tc.tile_pool(name="red", bufs=3) as rpool,
            tc.tile_pool(name="psE", bufs=2, space="PSUM") as psE,
            tc.tile_pool(name="psS", bufs=1, space="PSUM") as psS,
        ):
            # Split big inputs per-slot across the three DMA paths (SP/ACT
            # HWDGE + Pool SWDGE) so slot 0's data lands in ~3us instead of
            # waiting on one serialized ~11us queue.
            scal_t = []
            rep4_t = []
            offs_d = [0]
            for W in widths:
                offs_d.append(offs_d[-1] + W)
            for s, W in enumerate(widths):
                scal_t.append(cpool.tile([128, 32], f32, tag=f"scal{s}", name=f"scal{s}"))
                rep4_t.append(cpool.tile([128, W], bf16, tag=f"rep4{s}", name=f"rep4{s}"))
            # Slot 0's data first, on the earliest-available queues (SP for
            # the big window chunk, ACT for the scalars); later chunks go to
            # the Pool SWDGE / remaining SP slots so transfers overlap.
            h0 = widths[0] // 2
            nc.sync.dma_start(rep4_t[0][:, :h0], rep4_d[:, 0:h0])
            nc.scalar.dma_start(scal_t[0][:], scal_d[:, 0:32])
            nc.sync.dma_start(
                rep4_t[0][:, h0 : widths[0]], rep4_d[:, h0 : widths[0]]
            )
            if S > 1:
                nc.sync.dma_start(
                    rep4_t[1][:], rep4_d[:, offs_d[1] : offs_d[1] + widths[1]]
                )
                nc.scalar.dma_start(scal_t[1][:], scal_d[:, 32:64])
            for s, W in enumerate(widths):
                if s < 2:
                    continue
                nc.sync.dma_start(scal_t[s][:], scal_d[:, s * 32 : (s + 1) * 32])
                nc.gpsimd.dma_start(rep4_t[s][:], rep4_d[:, offs_d[s] : offs_d[s] + W])
            lhsw = cpool.tile([32, S * 128], bf16, tag="lhsw")
            nc.scalar.dma_start(lhsw[:], lhsw_d[:])
            onesbd = cpool.tile([128, 8 * 32], bf16, tag="onesbd")
            nc.scalar.dma_start(onesbd[:], ones_d[:])
            wsum = cpool.tile([32, S], bf16, tag="wsum")
            nc.sync.dma_start(wsum[:], wsum_d[:])
            ndw = cpool.tile([128, S], f32, tag="ndw")
            nc.sync.dma_start(ndw[:], ndw_d[:])
            acc = cpool.tile([128, 1], f32, tag="acc")
            nc.sync.dma_start(acc[:], acc0_d[:])

            sSw = psS.tile([128, 1], f32, tag="sSw")
            sLm = cpool.tile([128, S], f32, tag="sLm")

            def emit_ln(E, W, s):
                lnout = rpool.tile([128, W0], bf16, tag="lnout")
                nc.scalar.activation(
                    lnout[:, :W], E[:, :W], Ln, accum_out=sLm[:, s : s + 1]
                )

            def emit_muls(s, W):
                prod = wpool.tile([128, 32 * W0], bf16, tag="prod", name="prod")
                for k in range(32):
                    nc.vector.tensor_scalar(
                        prod[:, k * W : (k + 1) * W],
                        rep4_t[s][:],
                        scal_t[s][:, k : k + 1],
                        None,
                        op0=mult,
                    )
                return prod

            # The last slot's muls are hoisted early in the DVE stream so
            # the final exp never stalls on the init-bound tail muls.
            prods = {}
            pend = None
            for s, W in enumerate(widths):
                if s not in prods:
                    prods[s] = emit_muls(s, W)
                prod = prods.pop(s)
                # exp in two halves so the PE cascade starts at half-time;
                # the previous slot's ln is emitted after this exp so its
                # matmuls get covered by the exp run.
                expt = wpool.tile([128, 32 * W0], bf16, tag="expt")
                nc.scalar.activation(
                    expt[:, : 16 * W], prod[:, : 16 * W], Exp
                )
                nc.scalar.activation(
                    expt[:, 16 * W : 32 * W], prod[:, 16 * W : 32 * W], Exp
                )
                if pend is not None:
                    emit_ln(*pend)

                nc.tensor.matmul(
                    sSw[:],
                    lhsT=lhsw[:, s * 128 : (s + 1) * 128],
                    rhs=wsum[:, s : s + 1],
                    start=(s == 0),
                    stop=(s == S - 1),
                )

                E = psE.tile([128, W0], f32, tag="E")
                for k in range(32):
                    h, i = divmod(k, 8)
                    nc.tensor.matmul(
                        E[32 * h : 32 * h + 32, :W],
                        lhsT=onesbd[:, 32 * i : 32 * (i + 1)],
                        rhs=expt[:, k * W : (k + 1) * W],
                        start=(i == 0),
                        stop=(i == 7),
                        tile_position=(0, 32 * h),
                    )
                pend = (E, W, s)
            emit_ln(*pend)

            # res = acc0 + sSw + sum_s ndw[:,s] * sLm[:,s]
            wl = rpool.tile([128, S], f32, tag="wl")
            nc.vector.tensor_tensor(wl[:], sLm[:], ndw[:], mult)
            red = rpool.tile([128, 1], f32, tag="red")
            nc.vector.tensor_reduce(
                red[:], wl[:], axis=mybir.AxisListType.X, op=add
            )
            base = rpool.tile([128, 1], f32, tag="base")
            nc.vector.tensor_add(base[:], acc[:], sSw[:])
            res = rpool.tile([128, 1], f32, tag="res")
            nc.vector.tensor_add(res[:], base[:], red[:])
            nc.gpsimd.dma_start(out_d[:], res[:])

    nc.compile()
    return nc


def kernel(points, sensitive_attribute, t):
    _install_ntff_hook()
    _install_act_table_patch()

    points = np.asarray(points, dtype=np.float32)
    sa = np.asarray(sensitive_attribute).astype(np.int64)
    n, d = points.shape
    assert d == D

    scale = 1.0 / math.sqrt(float(np.asarray(t)))
    order = np.argsort(sa, kind="stable")
    sa_sorted = sa[order]
    ps = (points[order] * np.float32(scale)).astype(np.float32)  # [n, 32] sorted

    widths, per_core = _plan(sa_sorted)
    S = len(widths)
    CW = sum(widths)
    offs = np.concatenate([[0], np.cumsum(widths)]).astype(int)

    lnD = math.log(float(D))
    onesbd = np.zeros((128, 8 * 32), ml_dtypes.bfloat16)
    for i in range(8):
        for a in range(4):
            onesbd[32 * a : 32 * (a + 1), 36 * i + a] = 1.0

    in_maps = []
    for c in range(N_CORES):
        rep4 = np.zeros((128, CW), ml_dtypes.bfloat16)
        scal = np.zeros((128, S * 32), np.float32)
        lhsw = np.zeros((32, S * 128), ml_dtypes.bfloat16)
        wsum = np.zeros((32, S), ml_dtypes.bfloat16)
        ndw = np.zeros((128, S), np.float32)
        acc0 = np.zeros((128, 1), np.float32)
        for s, slot in enumerate(per_core[c]):
            if slot is None:
                continue  # dummy: zeros -> E=D, ln finite, weight 0
            a_lo, a_hi, g0, g1, c0, L, wfac = slot
            W = widths[s]
            m = a_hi - a_lo
            P = g1 - g0
            w = -wfac / (n * float(P) * float(P))
            win = ps[c0 : c0 + L].T  # [32, L]
            rep4[:, offs[s] : offs[s] + L] = np.tile(win, (4, 1))
            ablk = np.zeros((32, 128), np.float32)
            ablk[:, :m] = ps[a_lo:a_hi].T
            # scal col k = anchors 4k..4k+3 flattened (a-major, d-minor)
            scal[:, s * 32 : (s + 1) * 32] = ablk.T.reshape(32, 128).T
            lhsw[:, s * 128 : s * 128 + m] = np.float32(w) * ablk[:, :m]
            wsum[:, s] = win.sum(axis=1)
            ndw[:m, s] = -float(D) * w
            acc0[:m, 0] += w * D * lnD * (W - L)
        in_maps.append(
            {
                "rep4": rep4,
                "scal": scal,
                "lhsw": lhsw,
                "wsum": wsum,
                "ndw": ndw,
                "acc0": acc0,
                "onesbd": onesbd,
            }
        )

    key = tuple(widths)
    nc = _nc_cache.get(key)
    if nc is None:
        nc = _build_program(widths)
        _nc_cache[key] = nc
    trace = bool(int(os.environ.get("KERNEL_TRACE", "0")))
    res = run_bass_kernel_spmd(nc, in_maps, list(range(N_CORES)), trace=trace)
    last_run_info["exec_time_ns"] = res.exec_time_ns
    last_run_info["mean_exec_time_ns"] = res.mean_exec_time_ns
    last_run_info["W"] = widths
    last_run_info["ntiles"] = S
    last_run_info["instructions"] = (
        res.instructions_and_trace[0] if res.instructions_and_trace else None
    )

    total = 0.0
    for c in range(N_CORES):
        total += float(res.results[c]["out"].astype(np.float64).sum())
    return np.float32(total)
